# revision 21
# baseline (speedup 1.0000x reference)
"""ACM Graph Convolution on 8 TRN2 NeuronCores (Bass/Tile).

Strategy (dest-node sharded, per the sharding hint):
  - Each core owns N/8 destination rows.
  - Phase A: each core computes h_low/h_high = x_part @ W (bf16 TensorE),
    plus out_mlp = relu(x_part @ w_mlp) kept local.
  - AllGather h_low and h_high so each core holds the full [N, F_OUT]
    feature tables in local HBM (bf16 to halve traffic).
  - Phase C: edges are bucketed by (dest tile of 128 rows, source window)
    on the host, padded to 128-edge chunks.  For each chunk the device
    dma_gathers the 128 source rows, builds a one-hot*val mask on VectorE
    ((iota == dest_local) * val), and a TensorE matmul accumulates the
    segment sum into PSUM:  out[d,f] += sum_e mask[e,d] * h[col_e, f].
  - Epilogue per group of 4 dest tiles: relu, per-branch attention logits
    (VectorE reduce vs replicated av vectors), sigmoid -> 3x3 mix
    (att_vec baked as immediates) -> softmax -> weighted sum -> DMA out.

The graph is identical on all 8 cores (SPMD): chunk capacities are the
max over cores; shorter cores run padded chunks (val=0 -> no-op).
"""

import math

import numpy as np
import ml_dtypes

CORES = 8
P = 128
TG = 4  # dest tiles (of 128 rows) per PSUM group
FORCE_NWIN = None  # testing override for the source-window count

BF16 = ml_dtypes.bfloat16


# --------------------------------------------------------------------------
# Host-side edge preprocessing
# --------------------------------------------------------------------------

def _bucket_edges(row, col, val, n, n_per, t_tiles, n_win, win_rows):
    """Bucket edges by (dest core, dest tile, source window); stable sort.

    Returns (counts[c,t,w], per-bucket edge-id lists via sorted order),
    plus per-edge dest-in-tile r and window-rebased source index cr.
    """
    core = row // n_per
    dl = row - core * n_per
    t = dl // P
    r = (dl - t * P).astype(np.int32)
    w = col // win_rows
    cr = (col - w * win_rows).astype(np.int32)
    key = (core * t_tiles + t) * n_win + w
    order = np.argsort(key, kind="stable")
    counts = np.bincount(key, minlength=CORES * t_tiles * n_win).reshape(
        CORES, t_tiles, n_win
    )
    starts = np.zeros_like(counts)
    flat = counts.reshape(-1)
    st = np.concatenate([[0], np.cumsum(flat)[:-1]])
    starts = st.reshape(CORES, t_tiles, n_win)
    return counts, starts, order, r, cr


def preprocess(x, row_low, col_low, val_low, row_high, col_high, val_high,
               w_low, w_high, w_mlp, av_low, av_high, av_mlp, att_vec):
    n, f_in = x.shape
    f_out = w_low.shape[1]
    assert n % CORES == 0
    n_per = n // CORES
    t_tiles = (n_per + P - 1) // P
    n_win = FORCE_NWIN or (1 if n <= 32000 else int(math.ceil(n / 25000.0)))
    win_rows = int(math.ceil(n / n_win))

    groups = [list(range(i, min(i + TG, t_tiles))) for i in range(0, t_tiles, TG)]

    branches = []
    for (row, col, val) in ((row_low, col_low, val_low),
                            (row_high, col_high, val_high)):
        row = np.asarray(row).astype(np.int64)
        col = np.asarray(col).astype(np.int64)
        val = np.asarray(val).astype(np.float32)
        counts, starts, order, r, cr = _bucket_edges(
            row, col, val, n, n_per, t_tiles, n_win, win_rows)
        caps = (counts.max(axis=0) + P - 1) // P  # [t_tiles, n_win]
        # ensure every (t) has at least one chunk so PSUM gets zeroed
        for t in range(t_tiles):
            if caps[t].sum() == 0:
                caps[t][0] = 1
        branches.append(dict(counts=counts, starts=starts, order=order,
                             r=r, cr=cr, val=val, caps=caps))

    # ---- global chunk schedule (identical across cores) ----
    # per group: for b in (0,1): for w: for t in group: caps[b][t,w] chunks
    chunk_meta = []          # cid -> (b, t, w, k)
    schedule = []            # per group: dict(tiles=[...], segs=[...])
    chunk_off = {}           # (b, t, w) -> first cid
    for g_tiles in groups:
        segs = []
        for b in range(2):
            caps = branches[b]["caps"]
            for w in range(n_win):
                start_cid = len(chunk_meta)
                items = []
                for t in g_tiles:
                    chunk_off[(b, t, w)] = len(chunk_meta)
                    for k in range(int(caps[t, w])):
                        items.append((g_tiles.index(t), len(chunk_meta), t, k))
                        chunk_meta.append((b, t, w, k))
                s_chunks = len(chunk_meta) - start_cid
                if s_chunks:
                    segs.append(dict(b=b, w=w, off=start_cid, S=s_chunks,
                                     items=items))
        schedule.append(dict(tiles=g_tiles, segs=segs))
    nchunk = len(chunk_meta)

    # first/last chunk ids per (group, b): one PSUM accumulation group per
    # bank -- start_tensor_calc zero-marks the WHOLE 2KB bank, so per-tile
    # sub-groups within a shared bank must not interleave.
    first_chunk = {}
    last_chunk = {}
    for gi, grp in enumerate(schedule):
        for seg in grp["segs"]:
            for (ti, cid, t, k) in seg["items"]:
                key = (gi, seg["b"])
                if key not in first_chunk:
                    first_chunk[key] = cid
                last_chunk[key] = cid

    # ---- per-core slot arrays ----
    gidx_maps, dest_maps, val_maps = [], [], []
    for c in range(CORES):
        a_idx = np.zeros((nchunk, P), np.int16)
        a_r = np.zeros((nchunk, P), np.int16)
        a_v = np.zeros((nchunk, P), np.float32)
        for b in range(2):
            br = branches[b]
            for t in range(t_tiles):
                for w in range(n_win):
                    cnt = int(br["counts"][c, t, w])
                    if cnt == 0:
                        continue
                    st = int(br["starts"][c, t, w])
                    eids = br["order"][st:st + cnt]
                    off = chunk_off[(b, t, w)]
                    flat_i = a_idx[off:].reshape(-1)
                    flat_r = a_r[off:].reshape(-1)
                    flat_v = a_v[off:].reshape(-1)
                    flat_i[:cnt] = br["cr"][eids]
                    flat_r[:cnt] = br["r"][eids]
                    flat_v[:cnt] = br["val"][eids]
        gidx = a_idx.reshape(nchunk, 8, 16).transpose(2, 0, 1).reshape(16, nchunk * 8)
        gidx = np.tile(gidx, (8, 1))  # replicate to 128 partitions
        gidx_maps.append(np.ascontiguousarray(gidx))
        dest_maps.append(np.ascontiguousarray(a_r.T.astype(np.float32)))
        val_maps.append(np.ascontiguousarray(a_v.T))

    # ---- dense inputs ----
    xt = np.ascontiguousarray(np.asarray(x).astype(np.float32).T.astype(BF16))
    wcat = np.concatenate(
        [np.asarray(w).astype(np.float32) for w in (w_low, w_high, w_mlp)], axis=1
    ).astype(BF16)  # [f_in, 3*f_out]
    avrep = np.concatenate(
        [np.tile(np.asarray(a).astype(np.float32).reshape(1, f_out), (P, 1))
         for a in (av_low, av_high, av_mlp)], axis=1)  # [P, 3*f_out]
    iota = np.tile(np.arange(P, dtype=np.float32).astype(BF16), (P, 1))

    in_maps = []
    for c in range(CORES):
        in_maps.append({
            "xt": np.ascontiguousarray(xt[:, c * n_per:(c + 1) * n_per]),
            "wcat": wcat,
            "avrep": avrep.astype(np.float32),
            "iota": np.ascontiguousarray(iota),
            "gidx": gidx_maps[c],
            "destv": dest_maps[c],
            "valv": val_maps[c],
        })

    meta = dict(
        n=n, f_in=f_in, f_out=f_out, n_per=n_per, t_tiles=t_tiles,
        n_win=n_win, win_rows=win_rows, nchunk=nchunk,
        schedule=schedule, first_chunk=first_chunk, last_chunk=last_chunk,
        att=np.asarray(att_vec).astype(np.float64),
    )
    return meta, in_maps


# --------------------------------------------------------------------------
# Device graph
# --------------------------------------------------------------------------

def build_graph(meta, stage="full"):
    import concourse.bacc as bacc
    import concourse.tile as tile
    from concourse import library_config, mybir
    from concourse.tile_rust import add_dep_helper

    n = meta["n"]
    f_in = meta["f_in"]
    f_out = meta["f_out"]
    n_per = meta["n_per"]
    t_tiles = meta["t_tiles"]
    n_win = meta["n_win"]
    win_rows = meta["win_rows"]
    nchunk = meta["nchunk"]
    schedule = meta["schedule"]
    first_chunk = meta["first_chunk"]
    last_chunk = meta["last_chunk"]
    att = meta["att"]
    kc = f_in // P
    T = 3.0

    f32 = mybir.dt.float32
    bf16 = mybir.dt.bfloat16
    i16 = mybir.dt.int16

    nc = bacc.Bacc("TRN2", target_bir_lowering=False, debug=False,
                   num_devices=CORES)

    xt_p = nc.declare_dram_parameter("xt", [f_in, n_per], bf16, isOutput=False)
    wcat_p = nc.declare_dram_parameter("wcat", [f_in, 3 * f_out], bf16,
                                       isOutput=False)
    avrep_p = nc.declare_dram_parameter("avrep", [P, 3 * f_out], f32,
                                        isOutput=False)
    iota_p = nc.declare_dram_parameter("iota", [P, P], bf16, isOutput=False)
    gidx_p = nc.declare_dram_parameter("gidx", [P, nchunk * 8], i16,
                                       isOutput=False)
    dest_p = nc.declare_dram_parameter("destv", [P, nchunk], f32,
                                       isOutput=False)
    val_p = nc.declare_dram_parameter("valv", [P, nchunk], f32,
                                      isOutput=False)
    out_p = nc.declare_dram_parameter("out", [n_per, f_out], f32,
                                      isOutput=True)

    with tile.TileContext(nc) as tc:
        with tc.tile_pool(name="dram", bufs=1, space="DRAM") as dram_pool, \
             tc.tile_pool(name="static", bufs=1) as sp:
            hpart = [dram_pool.tile([n_per, f_out], bf16, name=f"hpart{b}")
                     for b in range(2)]
            hall = [dram_pool.tile([n, f_out], bf16, addr_space="Shared",
                                   name=f"hall{b}")
                    for b in range(2)]

            w_sb = sp.tile([P, kc, 3 * f_out], bf16, name="w_sb")
            av_sb = sp.tile([P, 3 * f_out], f32, name="av_sb")
            iota_sb = sp.tile([P, P], bf16, name="iota_sb")
            gidx_sb = sp.tile([P, nchunk * 8], i16, name="gidx_sb")
            dest_sb = sp.tile([P, nchunk], f32, name="dest_sb")
            val_sb = sp.tile([P, nchunk], f32, name="val_sb")
            omlp_sb = sp.tile([P, t_tiles * f_out], f32, name="omlp_sb")
            if n_per % P:
                nc.vector.memset(omlp_sb[:], 0)

            nc.sync.dma_start(out=w_sb[:],
                              in_=wcat_p[:].rearrange("(k p) f -> p k f", p=P))
            nc.sync.dma_start(out=av_sb[:], in_=avrep_p[:])
            nc.sync.dma_start(out=iota_sb[:], in_=iota_p[:])
            nc.sync.dma_start(out=gidx_sb[:], in_=gidx_p[:])
            nc.sync.dma_start(out=dest_sb[:], in_=dest_p[:])
            nc.sync.dma_start(out=val_sb[:], in_=val_p[:])

            # ---------------- Phase A: h = x @ W ----------------
            hpart_writes = []
            with tc.tile_pool(name="xtp", bufs=1) as xtp, \
                 tc.tile_pool(name="psA", bufs=2, space="PSUM") as psA, \
                 tc.tile_pool(name="hbp", bufs=3) as hbp:
                xt_sb = xtp.tile([P, kc, n_per], bf16, name="xt_sb")
                nc.sync.dma_start(
                    out=xt_sb[:],
                    in_=xt_p[:].rearrange("(k p) m -> p k m", p=P))
                for t in range(t_tiles):
                    m = min(P, n_per - t * P)
                    ph = psA.tile([P, 3 * f_out], f32, name="ph", tag="ph")
                    for k in range(kc):
                        nc.tensor.matmul(
                            out=ph[:m],
                            lhsT=xt_sb[:, k, t * P:t * P + m],
                            rhs=w_sb[:, k, :],
                            start=(k == 0), stop=(k == kc - 1))
                    hb = hbp.tile([P, 2 * f_out], bf16, name="hb", tag="hb")
                    nc.vector.tensor_copy(out=hb[:m], in_=ph[:m, 0:2 * f_out])
                    nc.scalar.activation(
                        out=omlp_sb[:m, t * f_out:(t + 1) * f_out],
                        in_=ph[:m, 2 * f_out:3 * f_out],
                        func=mybir.ActivationFunctionType.Relu)
                    d0 = nc.sync.dma_start(out=hpart[0][t * P:t * P + m, :],
                                           in_=hb[:m, 0:f_out])
                    d1 = nc.sync.dma_start(out=hpart[1][t * P:t * P + m, :],
                                           in_=hb[:m, f_out:2 * f_out])
                    hpart_writes += [d0, d1]

            if stage == "A":
                for t in range(t_tiles):
                    m = min(P, n_per - t * P)
                    nc.sync.dma_start(
                        out=out_p[t * P:t * P + m, :],
                        in_=omlp_sb[:m, t * f_out:(t + 1) * f_out])

            # ---------------- AllGather ----------------
            cc = []
            for b in range(2 if stage != "A" else 0):
                cci = nc.gpsimd.collective_compute(
                    "AllGather", mybir.AluOpType.bypass,
                    replica_groups=[list(range(CORES))],
                    ins=[hpart[b].opt()],
                    outs=[hall[b].opt()])
                for dw in hpart_writes:
                    add_dep_helper(cci.ins, dw.ins, True,
                                   reason="cc after hpart write")
                cc.append(cci)

            if stage == "AG":
                d = nc.gpsimd.dma_start(out=out_p[0:P, :],
                                        in_=hall[0][0:P, :])
                add_dep_helper(d.ins, cc[0].ins, True, reason="dbg")
                d = nc.gpsimd.dma_start(out=out_p[P:2 * P, :],
                                        in_=hall[1][0:P, :])
                add_dep_helper(d.ins, cc[1].ins, True, reason="dbg")

            # ---------------- Phase C: gather + segment-sum ----------------
            if stage in ("A", "AG"):
                schedule = []
            with tc.tile_pool(name="gbp", bufs=3) as gbp, \
                 tc.tile_pool(name="mkp", bufs=8) as mkp, \
                 tc.tile_pool(name="psC", bufs=2, space="PSUM") as psC, \
                 tc.tile_pool(name="epp", bufs=2) as epp:
                for gi, grp in enumerate(schedule):
                    tiles = grp["tiles"]
                    tloc = len(tiles)
                    ps = [psC.tile([P, TG * f_out], f32, name=f"ps{b}",
                                   tag=f"ps{b}") for b in range(2)]
                    for seg in grp["segs"]:
                        b, w, off, S = seg["b"], seg["w"], seg["off"], seg["S"]
                        gb = gbp.tile([P, S * f_out], bf16, name="gb", tag="gb")
                        gat = nc.gpsimd.dma_gather(
                            out_ap=gb[:].rearrange("p (s f) -> p s f", f=f_out),
                            in_ap=hall[b][w * win_rows:
                                          min((w + 1) * win_rows, n), :],
                            idxs_ap=gidx_sb[:, off * 8:(off + S) * 8],
                            num_idxs=S * P,
                            num_idxs_reg=S * P,
                            elem_size=f_out,
                            single_packet=False)
                        add_dep_helper(gat.ins, cc[b].ins, True,
                                       reason="gather after allgather")
                        if stage == "G":
                            nc.vector.tensor_copy(
                                out=ps[b][:, 0:f_out],
                                in_=gb[:, 0:f_out])
                            continue
                        for (ti, cid, t, k) in seg["items"]:
                            sl = cid - off
                            mk = mkp.tile([P, P], bf16, name="mk", tag="mk")
                            nc.vector.tensor_scalar(
                                out=mk[:], in0=iota_sb[:],
                                scalar1=dest_sb[:, cid:cid + 1],
                                scalar2=val_sb[:, cid:cid + 1],
                                op0=mybir.AluOpType.is_equal,
                                op1=mybir.AluOpType.mult)
                            nc.tensor.matmul(
                                out=ps[b][:, ti * f_out:(ti + 1) * f_out],
                                lhsT=mk[:],
                                rhs=gb[:, sl * f_out:(sl + 1) * f_out],
                                start=(cid == first_chunk[(gi, b)]),
                                stop=(cid == last_chunk[(gi, b)]),
                                skip_group_check=True)

                    # ---------------- epilogue ----------------
                    g0 = tiles[0]
                    fw = tloc * f_out
                    ol = epp.tile([P, TG * f_out], f32, name="ol", tag="ol")
                    oh = epp.tile([P, TG * f_out], f32, name="oh", tag="oh")
                    nc.scalar.activation(out=ol[:, :fw], in_=ps[0][:, :fw],
                                         func=mybir.ActivationFunctionType.Relu)
                    nc.scalar.activation(out=oh[:, :fw], in_=ps[1][:, :fw],
                                         func=mybir.ActivationFunctionType.Relu)
                    if stage in ("G", "M"):
                        for ti, t in enumerate(tiles):
                            m = min(P, n_per - t * P)
                            nc.sync.dma_start(
                                out=out_p[t * P:t * P + m, :],
                                in_=ol[:m, ti * f_out:(ti + 1) * f_out])
                        continue
                    om = omlp_sb[:, g0 * f_out:(g0 + tloc) * f_out]

                    tmp = epp.tile([P, TG * f_out], f32, name="tmp", tag="tmp")
                    lg = epp.tile([P, 3 * TG], f32, name="lg", tag="lg")
                    srcs = [ol[:, :fw], oh[:, :fw], om]
                    for j in range(3):
                        s3d = srcs[j].rearrange("p (t f) -> p t f", f=f_out)
                        a3d = av_sb[:, j * f_out:(j + 1) * f_out][:, None, :] \
                            .broadcast_to([P, tloc, f_out])
                        nc.vector.tensor_tensor(
                            out=tmp[:, :fw].rearrange("p (t f) -> p t f",
                                                      f=f_out),
                            in0=s3d, in1=a3d, op=mybir.AluOpType.mult)
                        nc.vector.tensor_reduce(
                            out=lg[:, j * tloc:(j + 1) * tloc],
                            in_=tmp[:, :fw].rearrange("p (t f) -> p t f",
                                                      f=f_out),
                            axis=mybir.AxisListType.X,
                            op=mybir.AluOpType.add)
                    sg = epp.tile([P, 3 * TG], f32, name="sg", tag="sg")
                    nc.scalar.activation(out=sg[:, :3 * tloc],
                                         in_=lg[:, :3 * tloc],
                                         func=mybir.ActivationFunctionType.Sigmoid)
                    zt = epp.tile([P, 3 * TG], f32, name="zt", tag="zt")
                    t2 = epp.tile([P, TG], f32, name="t2", tag="t2")
                    for j in range(3):
                        zj = zt[:, j * tloc:(j + 1) * tloc]
                        nc.vector.tensor_scalar(
                            out=zj, in0=sg[:, 0:tloc],
                            scalar1=float(att[0, j] / T), scalar2=None,
                            op0=mybir.AluOpType.mult)
                        for k2 in (1, 2):
                            nc.vector.tensor_scalar(
                                out=t2[:, :tloc],
                                in0=sg[:, k2 * tloc:(k2 + 1) * tloc],
                                scalar1=float(att[k2, j] / T), scalar2=None,
                                op0=mybir.AluOpType.mult)
                            nc.vector.tensor_tensor(
                                out=zj, in0=zj, in1=t2[:, :tloc],
                                op=mybir.AluOpType.add)
                    et = epp.tile([P, 3 * TG], f32, name="et", tag="et")
                    nc.scalar.activation(out=et[:, :3 * tloc],
                                         in_=zt[:, :3 * tloc],
                                         func=mybir.ActivationFunctionType.Exp)
                    s3 = epp.tile([P, TG], f32, name="s3", tag="s3")
                    nc.vector.tensor_tensor(out=s3[:, :tloc],
                                            in0=et[:, 0:tloc],
                                            in1=et[:, tloc:2 * tloc],
                                            op=mybir.AluOpType.add)
                    nc.vector.tensor_tensor(out=s3[:, :tloc],
                                            in0=s3[:, :tloc],
                                            in1=et[:, 2 * tloc:3 * tloc],
                                            op=mybir.AluOpType.add)
                    # rcp = 3/(sum e)  so that e*rcp = 3*att
                    nc.vector.reciprocal(out=s3[:, :tloc], in_=s3[:, :tloc])
                    nc.vector.tensor_scalar(out=s3[:, :tloc], in0=s3[:, :tloc],
                                            scalar1=3.0, scalar2=None,
                                            op0=mybir.AluOpType.mult)
                    at = epp.tile([P, 3 * TG], f32, name="at", tag="at")
                    for j in range(3):
                        nc.vector.tensor_tensor(
                            out=at[:, j * tloc:(j + 1) * tloc],
                            in0=et[:, j * tloc:(j + 1) * tloc],
                            in1=s3[:, :tloc], op=mybir.AluOpType.mult)
                    oo = epp.tile([P, TG * f_out], f32, name="oo", tag="oo")
                    for j, src in enumerate(srcs):
                        dst = oo if j == 0 else tmp
                        a3d = at[:, j * tloc:(j + 1) * tloc][:, :, None] \
                            .broadcast_to([P, tloc, f_out])
                        nc.vector.tensor_tensor(
                            out=dst[:, :fw].rearrange("p (t f) -> p t f",
                                                      f=f_out),
                            in0=src.rearrange("p (t f) -> p t f", f=f_out),
                            in1=a3d, op=mybir.AluOpType.mult)
                        if j > 0:
                            nc.vector.tensor_tensor(out=oo[:, :fw],
                                                    in0=oo[:, :fw],
                                                    in1=tmp[:, :fw],
                                                    op=mybir.AluOpType.add)
                    for ti, t in enumerate(tiles):
                        m = min(P, n_per - t * P)
                        nc.sync.dma_start(
                            out=out_p[t * P:t * P + m, :],
                            in_=oo[:m, ti * f_out:(ti + 1) * f_out])
    nc.compile()
    return nc


# --------------------------------------------------------------------------
# Entry point
# --------------------------------------------------------------------------

def _solve(inputs, trace=False):
    from concourse.bass_utils import run_bass_kernel_spmd

    meta, in_maps = preprocess(**inputs)
    nc = build_graph(meta)
    res = run_bass_kernel_spmd(nc, in_maps, core_ids=list(range(CORES)),
                               trace=trace)
    out = np.concatenate([res.results[c]["out"] for c in range(CORES)], axis=0)
    return out.astype(np.float32), res


def kernel(**inputs):
    out, _ = _solve(inputs, trace=False)
    return out


# revision 22
# speedup vs baseline: 1.6000x; 1.6000x over previous
"""ACM Graph Convolution on 8 TRN2 NeuronCores (Bass/Tile).

Strategy (dest-node sharded):
  - Each core owns N/8 destination rows.
  - Phase A: each core computes h_low/h_high = x_part @ W (bf16 TensorE),
    plus out_mlp = relu(x_part @ w_mlp) kept local.
  - AllGather h_low / h_high so each core holds the full [N, F_OUT]
    bf16 feature tables in local HBM.
  - Phase C: edges are bucketed by (dest tile of 128 rows, source window)
    on the host and padded to 128-edge chunks.  Per chunk the device
    dma_gathers the 128 source rows (4 SWDGE queues round-robin) and a
    TensorE matmul with a HOST-PRECOMPUTED one-hot*val mask accumulates
    the segment sum into PSUM: out[d,f] += sum_e mask[e,d]*h[col_e,f].
    One PSUM accumulation group per bank (start_tensor_calc zero-marks
    the whole 2KB bank).
  - relu per group into big bf16 SBUF buffers; the 3-way attention
    epilogue runs once at the end on [128, T*128] tensors.

The graph is identical on all 8 cores (SPMD): chunk capacities are the
max over cores; shorter cores run padded chunks (val=0 -> no-op).
"""

import math

import numpy as np
import ml_dtypes

CORES = 8
P = 128
TG = 4  # dest tiles (of 128 rows) per PSUM group
NQ = 4  # SWDGE queues for gather descriptor generation
FORCE_NWIN = None  # testing override for the source-window count

BF16 = ml_dtypes.bfloat16


# --------------------------------------------------------------------------
# Host-side edge preprocessing
# --------------------------------------------------------------------------

def _bucket_edges(row, col, val, n, n_per, t_tiles, n_win, win_rows):
    core = row // n_per
    dl = row - core * n_per
    t = dl // P
    r = (dl - t * P).astype(np.int32)
    w = col // win_rows
    cr = (col - w * win_rows).astype(np.int32)
    key = (core * t_tiles + t) * n_win + w
    order = np.argsort(key, kind="stable")
    counts = np.bincount(key, minlength=CORES * t_tiles * n_win).reshape(
        CORES, t_tiles, n_win
    )
    st = np.concatenate([[0], np.cumsum(counts.reshape(-1))[:-1]])
    starts = st.reshape(CORES, t_tiles, n_win)
    return counts, starts, order, r, cr


def preprocess(x, row_low, col_low, val_low, row_high, col_high, val_high,
               w_low, w_high, w_mlp, av_low, av_high, av_mlp, att_vec):
    n, f_in = x.shape
    f_out = w_low.shape[1]
    assert n % CORES == 0
    n_per = n // CORES
    t_tiles = (n_per + P - 1) // P
    n_win = FORCE_NWIN or (1 if n <= 32000 else int(math.ceil(n / 25000.0)))
    win_rows = int(math.ceil(n / n_win))

    groups = [list(range(i, min(i + TG, t_tiles)))
              for i in range(0, t_tiles, TG)]

    branches = []
    for (row, col, val) in ((row_low, col_low, val_low),
                            (row_high, col_high, val_high)):
        row = np.asarray(row).astype(np.int64)
        col = np.asarray(col).astype(np.int64)
        val = np.asarray(val).astype(np.float32)
        counts, starts, order, r, cr = _bucket_edges(
            row, col, val, n, n_per, t_tiles, n_win, win_rows)
        caps = (counts.max(axis=0) + P - 1) // P  # [t_tiles, n_win]
        for t in range(t_tiles):
            if caps[t].sum() == 0:
                caps[t][0] = 1
        branches.append(dict(counts=counts, starts=starts, order=order,
                             r=r, cr=cr, val=val, caps=caps))

    # ---- global chunk schedule (identical across cores) ----
    chunk_meta = []          # cid -> (b, t, w, k)
    schedule = []            # per group: dict(tiles=[...], segs=[...])
    chunk_off = {}           # (b, t, w) -> first cid
    for g_tiles in groups:
        segs = []
        for b in range(2):
            caps = branches[b]["caps"]
            for w in range(n_win):
                start_cid = len(chunk_meta)
                items = []
                for t in g_tiles:
                    chunk_off[(b, t, w)] = len(chunk_meta)
                    for k in range(int(caps[t, w])):
                        items.append((g_tiles.index(t), len(chunk_meta), t, k))
                        chunk_meta.append((b, t, w, k))
                s_chunks = len(chunk_meta) - start_cid
                if s_chunks:
                    segs.append(dict(b=b, w=w, off=start_cid, S=s_chunks,
                                     items=items))
        schedule.append(dict(tiles=g_tiles, segs=segs))
    nchunk = len(chunk_meta)

    # one PSUM accumulation group per (group, branch) bank
    first_chunk = {}
    last_chunk = {}
    for gi, grp in enumerate(schedule):
        for seg in grp["segs"]:
            for (ti, cid, t, k) in seg["items"]:
                key = (gi, seg["b"])
                if key not in first_chunk:
                    first_chunk[key] = cid
                last_chunk[key] = cid

    # ---- per-core slot arrays ----
    gidx_maps, mask_maps = [], []
    for c in range(CORES):
        a_idx = np.zeros((nchunk, P), np.int16)
        a_r = np.zeros((nchunk, P), np.int16)
        a_v = np.zeros((nchunk, P), np.float32)
        for b in range(2):
            br = branches[b]
            for t in range(t_tiles):
                for w in range(n_win):
                    cnt = int(br["counts"][c, t, w])
                    if cnt == 0:
                        continue
                    st = int(br["starts"][c, t, w])
                    eids = br["order"][st:st + cnt]
                    off = chunk_off[(b, t, w)]
                    a_idx[off:].reshape(-1)[:cnt] = br["cr"][eids]
                    a_r[off:].reshape(-1)[:cnt] = br["r"][eids]
                    a_v[off:].reshape(-1)[:cnt] = br["val"][eids]
        gidx = a_idx.reshape(nchunk, 8, 16).transpose(2, 0, 1)\
            .reshape(16, nchunk * 8)
        gidx = np.tile(gidx, (8, 1))
        gidx_maps.append(np.ascontiguousarray(gidx))
        # one-hot * val masks: M[ci, e, d] = (a_r[ci,e]==d) * a_v[ci,e]
        m = np.zeros((nchunk, P, P), BF16)
        ci = np.arange(nchunk)[:, None]
        ei = np.arange(P)[None, :]
        m[ci, ei, a_r] = a_v.astype(BF16)
        mask_maps.append(np.ascontiguousarray(
            m.transpose(1, 0, 2).reshape(P, nchunk * P)))

    # ---- dense inputs ----
    xt = np.ascontiguousarray(np.asarray(x).astype(np.float32).T.astype(BF16))
    wcat = np.concatenate(
        [np.asarray(w).astype(np.float32) for w in (w_low, w_high, w_mlp)],
        axis=1).astype(BF16)
    avrep = np.concatenate(
        [np.tile(np.asarray(a).astype(np.float32).reshape(1, f_out), (P, 1))
         for a in (av_low, av_high, av_mlp)], axis=1).astype(BF16)

    in_maps = []
    for c in range(CORES):
        in_maps.append({
            "xt": np.ascontiguousarray(xt[:, c * n_per:(c + 1) * n_per]),
            "wcat": wcat,
            "avrep": avrep,
            "gidx": gidx_maps[c],
            "masks": mask_maps[c],
        })

    meta = dict(
        n=n, f_in=f_in, f_out=f_out, n_per=n_per, t_tiles=t_tiles,
        n_win=n_win, win_rows=win_rows, nchunk=nchunk,
        schedule=schedule, first_chunk=first_chunk, last_chunk=last_chunk,
        att=np.asarray(att_vec).astype(np.float64),
    )
    return meta, in_maps


# --------------------------------------------------------------------------
# Device graph
# --------------------------------------------------------------------------

def build_graph(meta):
    import concourse.bacc as bacc
    import concourse.tile as tile
    from concourse import mybir
    from concourse.tile_rust import add_dep_helper

    n = meta["n"]
    f_in = meta["f_in"]
    f_out = meta["f_out"]
    n_per = meta["n_per"]
    t_tiles = meta["t_tiles"]
    n_win = meta["n_win"]
    win_rows = meta["win_rows"]
    nchunk = meta["nchunk"]
    schedule = meta["schedule"]
    first_chunk = meta["first_chunk"]
    last_chunk = meta["last_chunk"]
    att = meta["att"]
    kc = f_in // P
    T = 3.0
    TT = t_tiles

    f32 = mybir.dt.float32
    bf16 = mybir.dt.bfloat16
    i16 = mybir.dt.int16
    AF = mybir.ActivationFunctionType
    OP = mybir.AluOpType

    nc = bacc.Bacc("TRN2", target_bir_lowering=False, debug=False,
                   num_devices=CORES, num_swdge_queues=NQ)

    xt_p = nc.declare_dram_parameter("xt", [f_in, n_per], bf16,
                                     isOutput=False)
    wcat_p = nc.declare_dram_parameter("wcat", [f_in, 3 * f_out], bf16,
                                       isOutput=False)
    avrep_p = nc.declare_dram_parameter("avrep", [P, 3 * f_out], bf16,
                                        isOutput=False)
    gidx_p = nc.declare_dram_parameter("gidx", [P, nchunk * 8], i16,
                                       isOutput=False)
    mask_p = nc.declare_dram_parameter("masks", [P, nchunk * P], bf16,
                                       isOutput=False)
    out_p = nc.declare_dram_parameter("out", [n_per, f_out], f32,
                                      isOutput=True)

    with tile.TileContext(nc) as tc:
        with tc.tile_pool(name="dram", bufs=1, space="DRAM") as dram_pool, \
             tc.tile_pool(name="static", bufs=1) as sp:
            hpart = [dram_pool.tile([n_per, f_out], bf16, name=f"hpart{b}")
                     for b in range(2)]
            hall = [dram_pool.tile([n, f_out], bf16, addr_space="Shared",
                                   name=f"hall{b}")
                    for b in range(2)]

            w_sb = sp.tile([P, kc, 3 * f_out], bf16, name="w_sb")
            av_sb = sp.tile([P, 3 * f_out], bf16, name="av_sb")
            gidx_sb = sp.tile([P, nchunk * 8], i16, name="gidx_sb")
            olall = sp.tile([P, TT * f_out], bf16, name="olall")
            ohall = sp.tile([P, TT * f_out], bf16, name="ohall")
            omlp = sp.tile([P, TT * f_out], bf16, name="omlp")

            nc.sync.dma_start(out=w_sb[:],
                              in_=wcat_p[:].rearrange("(k p) f -> p k f",
                                                      p=P))
            nc.sync.dma_start(out=av_sb[:], in_=avrep_p[:])
            nc.sync.dma_start(out=gidx_sb[:], in_=gidx_p[:])
            if n_per % P:
                nc.vector.memset(omlp[:], 0)

            # ---------------- Phase A: h = x @ W ----------------
            hpart_writes = []
            with tc.tile_pool(name="xtp", bufs=1) as xtp, \
                 tc.tile_pool(name="psA", bufs=2, space="PSUM") as psA, \
                 tc.tile_pool(name="hbp", bufs=3) as hbp:
                xt_sb = xtp.tile([P, kc, n_per], bf16, name="xt_sb")
                nc.sync.dma_start(
                    out=xt_sb[:],
                    in_=xt_p[:].rearrange("(k p) m -> p k m", p=P))
                for t in range(t_tiles):
                    m = min(P, n_per - t * P)
                    ph = psA.tile([P, 3 * f_out], f32, name="ph", tag="ph")
                    for k in range(kc):
                        nc.tensor.matmul(
                            out=ph[:m],
                            lhsT=xt_sb[:, k, t * P:t * P + m],
                            rhs=w_sb[:, k, :],
                            start=(k == 0), stop=(k == kc - 1))
                    hb = hbp.tile([P, 2 * f_out], bf16, name="hb", tag="hb")
                    nc.vector.tensor_copy(out=hb[:m], in_=ph[:m, 0:2 * f_out])
                    nc.scalar.activation(
                        out=omlp[:m, t * f_out:(t + 1) * f_out],
                        in_=ph[:m, 2 * f_out:3 * f_out], func=AF.Relu)
                    d0 = nc.sync.dma_start(out=hpart[0][t * P:t * P + m, :],
                                           in_=hb[:m, 0:f_out])
                    d1 = nc.sync.dma_start(out=hpart[1][t * P:t * P + m, :],
                                           in_=hb[:m, f_out:2 * f_out])
                    hpart_writes += [d0, d1]

            # ---------------- AllGather ----------------
            cc = []
            for b in range(2):
                cci = nc.gpsimd.collective_compute(
                    "AllGather", OP.bypass,
                    replica_groups=[list(range(CORES))],
                    ins=[hpart[b].opt()],
                    outs=[hall[b].opt()])
                for dw in hpart_writes:
                    add_dep_helper(cci.ins, dw.ins, True,
                                   reason="cc after hpart write")
                cc.append(cci)

            # -------- Phase C: gather + mask-matmul segment sum --------
            qn = 0
            with tc.tile_pool(name="gbp", bufs=3) as gbp, \
                 tc.tile_pool(name="mkp", bufs=3) as mkp, \
                 tc.tile_pool(name="psC", bufs=2, space="PSUM") as psC:
                for gi, grp in enumerate(schedule):
                    tiles = grp["tiles"]
                    tloc = len(tiles)
                    g0 = tiles[0]
                    fw = tloc * f_out
                    ps = [psC.tile([P, TG * f_out], f32, name=f"ps{b}",
                                   tag=f"ps{b}") for b in range(2)]
                    for seg in grp["segs"]:
                        b, w, off, S = seg["b"], seg["w"], seg["off"], seg["S"]
                        gb = gbp.tile([P, S * f_out], bf16, name="gb",
                                      tag="gb")
                        gat = nc.gpsimd.dma_gather(
                            out_ap=gb[:].rearrange("p (s f) -> p s f",
                                                   f=f_out),
                            in_ap=hall[b][w * win_rows:
                                          min((w + 1) * win_rows, n), :],
                            idxs_ap=gidx_sb[:, off * 8:(off + S) * 8],
                            num_idxs=S * P,
                            num_idxs_reg=S * P,
                            elem_size=f_out,
                            single_packet=False,
                            queue_num=qn % NQ)
                        qn += 1
                        add_dep_helper(gat.ins, cc[b].ins, True,
                                       reason="gather after allgather")
                        mk = mkp.tile([P, S * f_out], bf16, name="mk",
                                      tag="mk")
                        nc.sync.dma_start(
                            out=mk[:],
                            in_=mask_p[:, off * P:(off + S) * P])
                        for (ti, cid, t, k) in seg["items"]:
                            sl = cid - off
                            nc.tensor.matmul(
                                out=ps[b][:, ti * f_out:(ti + 1) * f_out],
                                lhsT=mk[:, sl * P:(sl + 1) * P],
                                rhs=gb[:, sl * f_out:(sl + 1) * f_out],
                                start=(cid == first_chunk[(gi, b)]),
                                stop=(cid == last_chunk[(gi, b)]),
                                skip_group_check=True)
                    nc.scalar.activation(
                        out=olall[:, g0 * f_out:g0 * f_out + fw],
                        in_=ps[0][:, :fw], func=AF.Relu)
                    nc.scalar.activation(
                        out=ohall[:, g0 * f_out:g0 * f_out + fw],
                        in_=ps[1][:, :fw], func=AF.Relu)

            # ---------------- attention epilogue (batched) ----------------
            with tc.tile_pool(name="epp", bufs=1) as epp:
                tmp = epp.tile([P, TT * f_out], bf16, name="tmp")
                lg = epp.tile([P, 3 * TT], f32, name="lg")
                srcs = [olall, ohall, omlp]
                for j in range(3):
                    a3d = av_sb[:, j * f_out:(j + 1) * f_out][:, None, :] \
                        .broadcast_to([P, TT, f_out])
                    nc.vector.tensor_tensor(
                        out=tmp[:].rearrange("p (t f) -> p t f", f=f_out),
                        in0=srcs[j][:].rearrange("p (t f) -> p t f", f=f_out),
                        in1=a3d, op=OP.mult)
                    nc.vector.tensor_reduce(
                        out=lg[:, j * TT:(j + 1) * TT],
                        in_=tmp[:].rearrange("p (t f) -> p t f", f=f_out),
                        axis=mybir.AxisListType.X, op=OP.add)
                sg = epp.tile([P, 3 * TT], f32, name="sg")
                nc.scalar.activation(out=sg[:], in_=lg[:], func=AF.Sigmoid)
                zt = epp.tile([P, 3 * TT], f32, name="zt")
                t2 = epp.tile([P, TT], f32, name="t2")
                for j in range(3):
                    zj = zt[:, j * TT:(j + 1) * TT]
                    nc.vector.tensor_scalar(
                        out=zj, in0=sg[:, 0:TT],
                        scalar1=float(att[0, j] / T), scalar2=None,
                        op0=OP.mult)
                    for k2 in (1, 2):
                        nc.vector.tensor_scalar(
                            out=t2[:], in0=sg[:, k2 * TT:(k2 + 1) * TT],
                            scalar1=float(att[k2, j] / T), scalar2=None,
                            op0=OP.mult)
                        nc.vector.tensor_tensor(out=zj, in0=zj, in1=t2[:],
                                                op=OP.add)
                et = epp.tile([P, 3 * TT], f32, name="et")
                nc.scalar.activation(out=et[:], in_=zt[:], func=AF.Exp)
                s3 = epp.tile([P, TT], f32, name="s3")
                nc.vector.tensor_tensor(out=s3[:], in0=et[:, 0:TT],
                                        in1=et[:, TT:2 * TT], op=OP.add)
                nc.vector.tensor_tensor(out=s3[:], in0=s3[:],
                                        in1=et[:, 2 * TT:3 * TT], op=OP.add)
                # rcp = 3/(sum e)  so that e*rcp = 3*att
                nc.vector.reciprocal(out=s3[:], in_=s3[:])
                nc.vector.tensor_scalar(out=s3[:], in0=s3[:], scalar1=3.0,
                                        scalar2=None, op0=OP.mult)
                at = epp.tile([P, 3 * TT], bf16, name="at")
                for j in range(3):
                    nc.vector.tensor_tensor(
                        out=at[:, j * TT:(j + 1) * TT],
                        in0=et[:, j * TT:(j + 1) * TT],
                        in1=s3[:], op=OP.mult)
                oo = epp.tile([P, TT * f_out], f32, name="oo")
                tmp2 = epp.tile([P, TT * f_out], f32, name="tmp2")
                for j in range(3):
                    dst = oo if j == 0 else tmp2
                    a3d = at[:, j * TT:(j + 1) * TT][:, :, None] \
                        .broadcast_to([P, TT, f_out])
                    nc.vector.tensor_tensor(
                        out=dst[:].rearrange("p (t f) -> p t f", f=f_out),
                        in0=srcs[j][:].rearrange("p (t f) -> p t f", f=f_out),
                        in1=a3d, op=OP.mult)
                    if j > 0:
                        nc.vector.tensor_tensor(out=oo[:], in0=oo[:],
                                                in1=tmp2[:], op=OP.add)
                # output: full tiles in one 3D DMA, ragged tail separately
                nfull = n_per // P
                if nfull:
                    nc.sync.dma_start(
                        out=out_p[0:nfull * P, :].rearrange(
                            "(t p) f -> p t f", p=P),
                        in_=oo[:, 0:nfull * f_out].rearrange(
                            "p (t f) -> p t f", f=f_out))
                if n_per % P:
                    m = n_per - nfull * P
                    nc.sync.dma_start(
                        out=out_p[nfull * P:, :],
                        in_=oo[:m, nfull * f_out:(nfull + 1) * f_out])
    nc.compile()
    return nc


# --------------------------------------------------------------------------
# Entry point
# --------------------------------------------------------------------------

def _solve(inputs, trace=False):
    from concourse.bass_utils import run_bass_kernel_spmd

    meta, in_maps = preprocess(**inputs)
    nc = build_graph(meta)
    res = run_bass_kernel_spmd(nc, in_maps, core_ids=list(range(CORES)),
                               trace=trace)
    out = np.concatenate([res.results[c]["out"] for c in range(CORES)],
                         axis=0)
    return out.astype(np.float32), res


def kernel(**inputs):
    out, _ = _solve(inputs, trace=False)
    return out


# revision 24
# speedup vs baseline: 1.9388x; 1.2117x over previous
"""ACM Graph Convolution on 8 TRN2 NeuronCores (Bass/Tile).

Strategy (dest-node sharded):
  - Each core owns N/8 destination rows.
  - Phase A: each core computes h_low/h_high = x_part @ W (bf16 TensorE),
    plus out_mlp = relu(x_part @ w_mlp) kept local.
  - AllGather h_low / h_high so each core holds the full [N, F_OUT]
    bf16 feature tables in local HBM.
  - Phase C: edges are bucketed by (dest tile of 128 rows, source window)
    on the host and padded to 128-edge chunks.  Per chunk the device
    dma_gathers the 128 source rows (4 SWDGE queues round-robin) and a
    TensorE matmul with a HOST-PRECOMPUTED one-hot*val mask accumulates
    the segment sum into PSUM: out[d,f] += sum_e mask[e,d]*h[col_e,f].
    One PSUM accumulation group per bank (start_tensor_calc zero-marks
    the whole 2KB bank).
  - relu per group into big bf16 SBUF buffers; the 3-way attention
    epilogue runs once at the end on [128, T*128] tensors.

The graph is identical on all 8 cores (SPMD): chunk capacities are the
max over cores; shorter cores run padded chunks (val=0 -> no-op).
"""

import math

import numpy as np
import ml_dtypes

CORES = 8
P = 128
TG = 4  # dest tiles (of 128 rows) per PSUM group
NQ = 4  # SWDGE queues for gather descriptor generation
FORCE_NWIN = None  # testing override for the source-window count

BF16 = ml_dtypes.bfloat16


# --------------------------------------------------------------------------
# Host-side edge preprocessing
# --------------------------------------------------------------------------

def _bucket_edges(row, col, val, n, n_per, t_tiles, n_win, win_rows):
    core = row // n_per
    dl = row - core * n_per
    t = dl // P
    r = (dl - t * P).astype(np.int32)
    w = col // win_rows
    cr = (col - w * win_rows).astype(np.int32)
    key = (core * t_tiles + t) * n_win + w
    order = np.argsort(key, kind="stable")
    counts = np.bincount(key, minlength=CORES * t_tiles * n_win).reshape(
        CORES, t_tiles, n_win
    )
    st = np.concatenate([[0], np.cumsum(counts.reshape(-1))[:-1]])
    starts = st.reshape(CORES, t_tiles, n_win)
    return counts, starts, order, r, cr


def preprocess(x, row_low, col_low, val_low, row_high, col_high, val_high,
               w_low, w_high, w_mlp, av_low, av_high, av_mlp, att_vec):
    n, f_in = x.shape
    f_out = w_low.shape[1]
    assert n % CORES == 0
    n_per = n // CORES
    t_tiles = (n_per + P - 1) // P
    n_win = FORCE_NWIN or (1 if n <= 32000 else int(math.ceil(n / 25000.0)))
    win_rows = int(math.ceil(n / n_win))

    groups = [list(range(i, min(i + TG, t_tiles)))
              for i in range(0, t_tiles, TG)]

    branches = []
    for (row, col, val) in ((row_low, col_low, val_low),
                            (row_high, col_high, val_high)):
        row = np.asarray(row).astype(np.int64)
        col = np.asarray(col).astype(np.int64)
        val = np.asarray(val).astype(np.float32)
        counts, starts, order, r, cr = _bucket_edges(
            row, col, val, n, n_per, t_tiles, n_win, win_rows)
        caps = (counts.max(axis=0) + P - 1) // P  # [t_tiles, n_win]
        for t in range(t_tiles):
            if caps[t].sum() == 0:
                caps[t][0] = 1
        branches.append(dict(counts=counts, starts=starts, order=order,
                             r=r, cr=cr, val=val, caps=caps))

    # ---- global chunk schedule (identical across cores) ----
    chunk_meta = []          # cid -> (b, t, w, k)
    schedule = []            # per group: dict(tiles=[...], segs=[...])
    chunk_off = {}           # (b, t, w) -> first cid
    for g_tiles in groups:
        segs = []
        for b in range(2):
            caps = branches[b]["caps"]
            for w in range(n_win):
                start_cid = len(chunk_meta)
                items = []
                for t in g_tiles:
                    chunk_off[(b, t, w)] = len(chunk_meta)
                    for k in range(int(caps[t, w])):
                        items.append((g_tiles.index(t), len(chunk_meta), t, k))
                        chunk_meta.append((b, t, w, k))
                s_chunks = len(chunk_meta) - start_cid
                if s_chunks:
                    segs.append(dict(b=b, w=w, off=start_cid, S=s_chunks,
                                     items=items))
        schedule.append(dict(tiles=g_tiles, segs=segs))
    nchunk = len(chunk_meta)

    # one PSUM accumulation group per (group, branch) bank
    first_chunk = {}
    last_chunk = {}
    for gi, grp in enumerate(schedule):
        for seg in grp["segs"]:
            for (ti, cid, t, k) in seg["items"]:
                key = (gi, seg["b"])
                if key not in first_chunk:
                    first_chunk[key] = cid
                last_chunk[key] = cid

    # ---- per-core slot arrays ----
    gidx_maps, mask_maps = [], []
    for c in range(CORES):
        a_idx = np.zeros((nchunk, P), np.int16)
        a_r = np.zeros((nchunk, P), np.int16)
        a_v = np.zeros((nchunk, P), np.float32)
        for b in range(2):
            br = branches[b]
            for t in range(t_tiles):
                for w in range(n_win):
                    cnt = int(br["counts"][c, t, w])
                    if cnt == 0:
                        continue
                    st = int(br["starts"][c, t, w])
                    eids = br["order"][st:st + cnt]
                    off = chunk_off[(b, t, w)]
                    a_idx[off:].reshape(-1)[:cnt] = br["cr"][eids]
                    a_r[off:].reshape(-1)[:cnt] = br["r"][eids]
                    a_v[off:].reshape(-1)[:cnt] = br["val"][eids]
        gidx = a_idx.reshape(nchunk, 8, 16).transpose(2, 0, 1)\
            .reshape(16, nchunk * 8)
        gidx = np.tile(gidx, (8, 1))
        gidx_maps.append(np.ascontiguousarray(gidx))
        # one-hot * val masks: M[ci, e, d] = (a_r[ci,e]==d) * a_v[ci,e]
        m = np.zeros((nchunk, P, P), BF16)
        ci = np.arange(nchunk)[:, None]
        ei = np.arange(P)[None, :]
        m[ci, ei, a_r] = a_v.astype(BF16)
        mask_maps.append(np.ascontiguousarray(
            m.transpose(1, 0, 2).reshape(P, nchunk * P)))

    # ---- dense inputs ----
    xt = np.ascontiguousarray(np.asarray(x).astype(np.float32).T.astype(BF16))
    wcat = np.concatenate(
        [np.asarray(w).astype(np.float32) for w in (w_low, w_high, w_mlp)],
        axis=1).astype(BF16)
    avrep = np.concatenate(
        [np.tile(np.asarray(a).astype(np.float32).reshape(1, f_out), (P, 1))
         for a in (av_low, av_high, av_mlp)], axis=1).astype(BF16)

    in_maps = []
    for c in range(CORES):
        in_maps.append({
            "xt": np.ascontiguousarray(xt[:, c * n_per:(c + 1) * n_per]),
            "wcat": wcat,
            "avrep": avrep,
            "gidx": gidx_maps[c],
            "masks": mask_maps[c],
        })

    meta = dict(
        n=n, f_in=f_in, f_out=f_out, n_per=n_per, t_tiles=t_tiles,
        n_win=n_win, win_rows=win_rows, nchunk=nchunk,
        schedule=schedule, first_chunk=first_chunk, last_chunk=last_chunk,
        att=np.asarray(att_vec).astype(np.float64),
    )
    return meta, in_maps


# --------------------------------------------------------------------------
# Device graph
# --------------------------------------------------------------------------

def build_graph(meta):
    import concourse.bacc as bacc
    import concourse.tile as tile
    from concourse import mybir
    from concourse.tile_rust import add_dep_helper

    n = meta["n"]
    f_in = meta["f_in"]
    f_out = meta["f_out"]
    n_per = meta["n_per"]
    t_tiles = meta["t_tiles"]
    n_win = meta["n_win"]
    win_rows = meta["win_rows"]
    nchunk = meta["nchunk"]
    schedule = meta["schedule"]
    first_chunk = meta["first_chunk"]
    last_chunk = meta["last_chunk"]
    att = meta["att"]
    kc = f_in // P
    T = 3.0
    TT = t_tiles

    f32 = mybir.dt.float32
    bf16 = mybir.dt.bfloat16
    i16 = mybir.dt.int16
    AF = mybir.ActivationFunctionType
    OP = mybir.AluOpType

    nc = bacc.Bacc("TRN2", target_bir_lowering=False, debug=False,
                   num_devices=CORES, num_swdge_queues=NQ)

    xt_p = nc.declare_dram_parameter("xt", [f_in, n_per], bf16,
                                     isOutput=False)
    wcat_p = nc.declare_dram_parameter("wcat", [f_in, 3 * f_out], bf16,
                                       isOutput=False)
    avrep_p = nc.declare_dram_parameter("avrep", [P, 3 * f_out], bf16,
                                        isOutput=False)
    gidx_p = nc.declare_dram_parameter("gidx", [P, nchunk * 8], i16,
                                       isOutput=False)
    mask_p = nc.declare_dram_parameter("masks", [P, nchunk * P], bf16,
                                       isOutput=False)
    out_p = nc.declare_dram_parameter("out", [n_per, f_out], f32,
                                      isOutput=True)

    with tile.TileContext(nc) as tc:
        with tc.tile_pool(name="dram", bufs=1, space="DRAM") as dram_pool, \
             tc.tile_pool(name="static", bufs=1) as sp:
            hpart = [dram_pool.tile([n_per, f_out], bf16, name=f"hpart{b}")
                     for b in range(2)]
            hall = [dram_pool.tile([n, f_out], bf16, addr_space="Shared",
                                   name=f"hall{b}")
                    for b in range(2)]

            w_sb = sp.tile([P, kc, 3 * f_out], bf16, name="w_sb")
            av_sb = sp.tile([P, 3 * f_out], bf16, name="av_sb")
            gidx_sb = sp.tile([P, nchunk * 8], i16, name="gidx_sb")
            olall = sp.tile([P, TT * f_out], bf16, name="olall")
            ohall = sp.tile([P, TT * f_out], bf16, name="ohall")
            omlp = sp.tile([P, TT * f_out], bf16, name="omlp")

            nc.sync.dma_start(out=w_sb[:],
                              in_=wcat_p[:].rearrange("(k p) f -> p k f",
                                                      p=P))
            nc.sync.dma_start(out=av_sb[:], in_=avrep_p[:])
            nc.sync.dma_start(out=gidx_sb[:], in_=gidx_p[:])
            if n_per % P:
                nc.vector.memset(omlp[:], 0)

            # ---------------- Phase A: h = x @ W ----------------
            hpart_writes = []
            with tc.tile_pool(name="xtp", bufs=1) as xtp, \
                 tc.tile_pool(name="psA", bufs=2, space="PSUM") as psA, \
                 tc.tile_pool(name="hbp", bufs=3) as hbp:
                xt_sb = xtp.tile([P, kc, n_per], bf16, name="xt_sb")
                nc.sync.dma_start(
                    out=xt_sb[:],
                    in_=xt_p[:].rearrange("(k p) m -> p k m", p=P))
                for t in range(t_tiles):
                    m = min(P, n_per - t * P)
                    ph = psA.tile([P, 3 * f_out], f32, name="ph", tag="ph")
                    for k in range(kc):
                        nc.tensor.matmul(
                            out=ph[:m],
                            lhsT=xt_sb[:, k, t * P:t * P + m],
                            rhs=w_sb[:, k, :],
                            start=(k == 0), stop=(k == kc - 1))
                    hb = hbp.tile([P, 2 * f_out], bf16, name="hb", tag="hb")
                    nc.vector.tensor_copy(out=hb[:m], in_=ph[:m, 0:2 * f_out])
                    nc.scalar.activation(
                        out=omlp[:m, t * f_out:(t + 1) * f_out],
                        in_=ph[:m, 2 * f_out:3 * f_out], func=AF.Relu)
                    d0 = nc.sync.dma_start(out=hpart[0][t * P:t * P + m, :],
                                           in_=hb[:m, 0:f_out])
                    d1 = nc.sync.dma_start(out=hpart[1][t * P:t * P + m, :],
                                           in_=hb[:m, f_out:2 * f_out])
                    hpart_writes += [d0, d1]

            # ---------------- AllGather ----------------
            cc = []
            for b in range(2):
                cci = nc.gpsimd.collective_compute(
                    "AllGather", OP.bypass,
                    replica_groups=[list(range(CORES))],
                    ins=[hpart[b].opt()],
                    outs=[hall[b].opt()])
                for dw in hpart_writes:
                    add_dep_helper(cci.ins, dw.ins, True,
                                   reason="cc after hpart write")
                cc.append(cci)

            # -------- Phase C: gather + mask-matmul segment sum --------
            qn = 0
            with tc.tile_pool(name="gbp", bufs=5) as gbp, \
                 tc.tile_pool(name="mkp", bufs=5) as mkp, \
                 tc.tile_pool(name="psC", bufs=2, space="PSUM") as psC:
                for gi, grp in enumerate(schedule):
                    tiles = grp["tiles"]
                    tloc = len(tiles)
                    g0 = tiles[0]
                    fw = tloc * f_out
                    ps = [psC.tile([P, TG * f_out], f32, name=f"ps{b}",
                                   tag=f"ps{b}") for b in range(2)]
                    for seg in grp["segs"]:
                        b, w, off, S = seg["b"], seg["w"], seg["off"], seg["S"]
                        gb = gbp.tile([P, S * f_out], bf16, name="gb",
                                      tag="gb")
                        gat = nc.gpsimd.dma_gather(
                            out_ap=gb[:].rearrange("p (s f) -> p s f",
                                                   f=f_out),
                            in_ap=hall[b][w * win_rows:
                                          min((w + 1) * win_rows, n), :],
                            idxs_ap=gidx_sb[:, off * 8:(off + S) * 8],
                            num_idxs=S * P,
                            num_idxs_reg=S * P,
                            elem_size=f_out,
                            single_packet=False,
                            queue_num=qn % NQ)
                        qn += 1
                        add_dep_helper(gat.ins, cc[b].ins, True,
                                       reason="gather after allgather")
                        mk = mkp.tile([P, S * f_out], bf16, name="mk",
                                      tag="mk")
                        nc.sync.dma_start(
                            out=mk[:],
                            in_=mask_p[:, off * P:(off + S) * P])
                        for (ti, cid, t, k) in seg["items"]:
                            sl = cid - off
                            nc.tensor.matmul(
                                out=ps[b][:, ti * f_out:(ti + 1) * f_out],
                                lhsT=mk[:, sl * P:(sl + 1) * P],
                                rhs=gb[:, sl * f_out:(sl + 1) * f_out],
                                start=(cid == first_chunk[(gi, b)]),
                                stop=(cid == last_chunk[(gi, b)]),
                                skip_group_check=True)
                    nc.scalar.activation(
                        out=olall[:, g0 * f_out:g0 * f_out + fw],
                        in_=ps[0][:, :fw], func=AF.Relu)
                    nc.scalar.activation(
                        out=ohall[:, g0 * f_out:g0 * f_out + fw],
                        in_=ps[1][:, :fw], func=AF.Relu)

            # ---------------- attention epilogue (batched) ----------------
            with tc.tile_pool(name="epp", bufs=1) as epp:
                tmp = epp.tile([P, TT * f_out], bf16, name="tmp")
                lg = epp.tile([P, 3 * TT], f32, name="lg")
                srcs = [olall, ohall, omlp]
                for j in range(3):
                    a3d = av_sb[:, j * f_out:(j + 1) * f_out][:, None, :] \
                        .broadcast_to([P, TT, f_out])
                    nc.vector.tensor_tensor(
                        out=tmp[:].rearrange("p (t f) -> p t f", f=f_out),
                        in0=srcs[j][:].rearrange("p (t f) -> p t f", f=f_out),
                        in1=a3d, op=OP.mult)
                    nc.vector.tensor_reduce(
                        out=lg[:, j * TT:(j + 1) * TT],
                        in_=tmp[:].rearrange("p (t f) -> p t f", f=f_out),
                        axis=mybir.AxisListType.X, op=OP.add)
                sg = epp.tile([P, 3 * TT], f32, name="sg")
                nc.scalar.activation(out=sg[:], in_=lg[:], func=AF.Sigmoid)
                zt = epp.tile([P, 3 * TT], f32, name="zt")
                t2 = epp.tile([P, TT], f32, name="t2")
                for j in range(3):
                    zj = zt[:, j * TT:(j + 1) * TT]
                    nc.vector.tensor_scalar(
                        out=zj, in0=sg[:, 0:TT],
                        scalar1=float(att[0, j] / T), scalar2=None,
                        op0=OP.mult)
                    for k2 in (1, 2):
                        nc.vector.tensor_scalar(
                            out=t2[:], in0=sg[:, k2 * TT:(k2 + 1) * TT],
                            scalar1=float(att[k2, j] / T), scalar2=None,
                            op0=OP.mult)
                        nc.vector.tensor_tensor(out=zj, in0=zj, in1=t2[:],
                                                op=OP.add)
                et = epp.tile([P, 3 * TT], f32, name="et")
                nc.scalar.activation(out=et[:], in_=zt[:], func=AF.Exp)
                s3 = epp.tile([P, TT], f32, name="s3")
                nc.vector.tensor_tensor(out=s3[:], in0=et[:, 0:TT],
                                        in1=et[:, TT:2 * TT], op=OP.add)
                nc.vector.tensor_tensor(out=s3[:], in0=s3[:],
                                        in1=et[:, 2 * TT:3 * TT], op=OP.add)
                # rcp = 3/(sum e)  so that e*rcp = 3*att
                nc.vector.reciprocal(out=s3[:], in_=s3[:])
                nc.vector.tensor_scalar(out=s3[:], in0=s3[:], scalar1=3.0,
                                        scalar2=None, op0=OP.mult)
                at = epp.tile([P, 3 * TT], bf16, name="at")
                for j in range(3):
                    nc.vector.tensor_tensor(
                        out=at[:, j * TT:(j + 1) * TT],
                        in0=et[:, j * TT:(j + 1) * TT],
                        in1=s3[:], op=OP.mult)
                oo = epp.tile([P, TT * f_out], f32, name="oo")
                tmp2 = epp.tile([P, TT * f_out], f32, name="tmp2")
                for j in range(3):
                    dst = oo if j == 0 else tmp2
                    a3d = at[:, j * TT:(j + 1) * TT][:, :, None] \
                        .broadcast_to([P, TT, f_out])
                    nc.vector.tensor_tensor(
                        out=dst[:].rearrange("p (t f) -> p t f", f=f_out),
                        in0=srcs[j][:].rearrange("p (t f) -> p t f", f=f_out),
                        in1=a3d, op=OP.mult)
                    if j > 0:
                        nc.vector.tensor_tensor(out=oo[:], in0=oo[:],
                                                in1=tmp2[:], op=OP.add)
                # output: full tiles in one 3D DMA, ragged tail separately
                nfull = n_per // P
                if nfull:
                    nc.sync.dma_start(
                        out=out_p[0:nfull * P, :].rearrange(
                            "(t p) f -> p t f", p=P),
                        in_=oo[:, 0:nfull * f_out].rearrange(
                            "p (t f) -> p t f", f=f_out))
                if n_per % P:
                    m = n_per - nfull * P
                    nc.sync.dma_start(
                        out=out_p[nfull * P:, :],
                        in_=oo[:m, nfull * f_out:(nfull + 1) * f_out])
    nc.compile()
    return nc


# --------------------------------------------------------------------------
# Entry point
# --------------------------------------------------------------------------

def _solve(inputs, trace=False):
    from concourse.bass_utils import run_bass_kernel_spmd

    meta, in_maps = preprocess(**inputs)
    nc = build_graph(meta)
    res = run_bass_kernel_spmd(nc, in_maps, core_ids=list(range(CORES)),
                               trace=trace)
    out = np.concatenate([res.results[c]["out"] for c in range(CORES)],
                         axis=0)
    return out.astype(np.float32), res


def kernel(**inputs):
    out, _ = _solve(inputs, trace=False)
    return out


# revision 25
# speedup vs baseline: 2.1156x; 1.0912x over previous
"""ACM Graph Convolution on 8 TRN2 NeuronCores (Bass/Tile).

Strategy (dest-node sharded):
  - Each core owns N/8 destination rows.
  - Phase A: each core computes h_low/h_high = x_part @ W (bf16 TensorE),
    plus out_mlp = relu(x_part @ w_mlp) kept local.
  - AllGather h_low / h_high so each core holds the full [N, F_OUT]
    bf16 feature tables in local HBM.
  - Phase C: edges are bucketed by (dest tile of 128 rows, source window)
    on the host and padded to 128-edge chunks.  Per chunk the device
    dma_gathers the 128 source rows (4 SWDGE queues round-robin) and a
    TensorE matmul with a HOST-PRECOMPUTED one-hot*val mask accumulates
    the segment sum into PSUM: out[d,f] += sum_e mask[e,d]*h[col_e,f].
    One PSUM accumulation group per bank (start_tensor_calc zero-marks
    the whole 2KB bank).
  - relu per group into big bf16 SBUF buffers; the 3-way attention
    epilogue runs once at the end on [128, T*128] tensors.

The graph is identical on all 8 cores (SPMD): chunk capacities are the
max over cores; shorter cores run padded chunks (val=0 -> no-op).
"""

import math

import numpy as np
import ml_dtypes

CORES = 8
P = 128
TG = 4  # dest tiles (of 128 rows) per PSUM group
NQ = 4  # SWDGE queues for gather descriptor generation
FORCE_NWIN = None  # testing override for the source-window count

BF16 = ml_dtypes.bfloat16


# --------------------------------------------------------------------------
# Host-side edge preprocessing
# --------------------------------------------------------------------------

def _bucket_edges(row, col, val, n, n_per, t_tiles, n_win, win_rows):
    core = row // n_per
    dl = row - core * n_per
    t = dl // P
    r = (dl - t * P).astype(np.int32)
    w = col // win_rows
    cr = (col - w * win_rows).astype(np.int32)
    key = (core * t_tiles + t) * n_win + w
    order = np.argsort(key, kind="stable")
    counts = np.bincount(key, minlength=CORES * t_tiles * n_win).reshape(
        CORES, t_tiles, n_win
    )
    st = np.concatenate([[0], np.cumsum(counts.reshape(-1))[:-1]])
    starts = st.reshape(CORES, t_tiles, n_win)
    return counts, starts, order, r, cr


def preprocess(x, row_low, col_low, val_low, row_high, col_high, val_high,
               w_low, w_high, w_mlp, av_low, av_high, av_mlp, att_vec):
    n, f_in = x.shape
    f_out = w_low.shape[1]
    assert n % CORES == 0
    n_per = n // CORES
    t_tiles = (n_per + P - 1) // P
    n_win = FORCE_NWIN or (1 if n <= 32000 else int(math.ceil(n / 25000.0)))
    win_rows = int(math.ceil(n / n_win))

    groups = [list(range(i, min(i + TG, t_tiles)))
              for i in range(0, t_tiles, TG)]

    branches = []
    for (row, col, val) in ((row_low, col_low, val_low),
                            (row_high, col_high, val_high)):
        row = np.asarray(row).astype(np.int64)
        col = np.asarray(col).astype(np.int64)
        val = np.asarray(val).astype(np.float32)
        counts, starts, order, r, cr = _bucket_edges(
            row, col, val, n, n_per, t_tiles, n_win, win_rows)
        caps = (counts.max(axis=0) + P - 1) // P  # [t_tiles, n_win]
        for t in range(t_tiles):
            if caps[t].sum() == 0:
                caps[t][0] = 1
        branches.append(dict(counts=counts, starts=starts, order=order,
                             r=r, cr=cr, val=val, caps=caps))

    # ---- global chunk schedule (identical across cores) ----
    chunk_meta = []          # cid -> (b, t, w, k)
    schedule = []            # per group: dict(tiles=[...], segs=[...])
    chunk_off = {}           # (b, t, w) -> first cid
    for g_tiles in groups:
        segs = []
        for b in range(2):
            caps = branches[b]["caps"]
            for w in range(n_win):
                start_cid = len(chunk_meta)
                items = []
                for t in g_tiles:
                    chunk_off[(b, t, w)] = len(chunk_meta)
                    for k in range(int(caps[t, w])):
                        items.append((g_tiles.index(t), len(chunk_meta), t, k))
                        chunk_meta.append((b, t, w, k))
                s_chunks = len(chunk_meta) - start_cid
                if s_chunks:
                    segs.append(dict(b=b, w=w, off=start_cid, S=s_chunks,
                                     items=items))
        schedule.append(dict(tiles=g_tiles, segs=segs))
    nchunk = len(chunk_meta)

    # one PSUM accumulation group per (group, branch) bank
    first_chunk = {}
    last_chunk = {}
    for gi, grp in enumerate(schedule):
        for seg in grp["segs"]:
            for (ti, cid, t, k) in seg["items"]:
                key = (gi, seg["b"])
                if key not in first_chunk:
                    first_chunk[key] = cid
                last_chunk[key] = cid

    # ---- per-core slot arrays ----
    gidx_maps, mask_maps = [], []
    for c in range(CORES):
        a_idx = np.zeros((nchunk, P), np.int16)
        a_r = np.zeros((nchunk, P), np.int16)
        a_v = np.zeros((nchunk, P), np.float32)
        for b in range(2):
            br = branches[b]
            for t in range(t_tiles):
                for w in range(n_win):
                    cnt = int(br["counts"][c, t, w])
                    if cnt == 0:
                        continue
                    st = int(br["starts"][c, t, w])
                    eids = br["order"][st:st + cnt]
                    # ascending source order -> near-sequential HBM reads
                    eids = eids[np.argsort(br["cr"][eids], kind="stable")]
                    off = chunk_off[(b, t, w)]
                    a_idx[off:].reshape(-1)[:cnt] = br["cr"][eids]
                    a_r[off:].reshape(-1)[:cnt] = br["r"][eids]
                    a_v[off:].reshape(-1)[:cnt] = br["val"][eids]
        gidx = a_idx.reshape(nchunk, 8, 16).transpose(2, 0, 1)\
            .reshape(16, nchunk * 8)
        gidx = np.tile(gidx, (8, 1))
        gidx_maps.append(np.ascontiguousarray(gidx))
        # one-hot * val masks: M[ci, e, d] = (a_r[ci,e]==d) * a_v[ci,e]
        m = np.zeros((nchunk, P, P), BF16)
        ci = np.arange(nchunk)[:, None]
        ei = np.arange(P)[None, :]
        m[ci, ei, a_r] = a_v.astype(BF16)
        mask_maps.append(np.ascontiguousarray(
            m.transpose(1, 0, 2).reshape(P, nchunk * P)))

    # ---- dense inputs ----
    xt = np.ascontiguousarray(np.asarray(x).astype(np.float32).T.astype(BF16))
    wcat = np.concatenate(
        [np.asarray(w).astype(np.float32) for w in (w_low, w_high, w_mlp)],
        axis=1).astype(BF16)
    avrep = np.concatenate(
        [np.tile(np.asarray(a).astype(np.float32).reshape(1, f_out), (P, 1))
         for a in (av_low, av_high, av_mlp)], axis=1).astype(BF16)

    in_maps = []
    for c in range(CORES):
        in_maps.append({
            "xt": np.ascontiguousarray(xt[:, c * n_per:(c + 1) * n_per]),
            "wcat": wcat,
            "avrep": avrep,
            "gidx": gidx_maps[c],
            "masks": mask_maps[c],
        })

    meta = dict(
        n=n, f_in=f_in, f_out=f_out, n_per=n_per, t_tiles=t_tiles,
        n_win=n_win, win_rows=win_rows, nchunk=nchunk,
        schedule=schedule, first_chunk=first_chunk, last_chunk=last_chunk,
        att=np.asarray(att_vec).astype(np.float64),
    )
    return meta, in_maps


# --------------------------------------------------------------------------
# Device graph
# --------------------------------------------------------------------------

def build_graph(meta):
    import concourse.bacc as bacc
    import concourse.tile as tile
    from concourse import mybir
    from concourse.tile_rust import add_dep_helper

    n = meta["n"]
    f_in = meta["f_in"]
    f_out = meta["f_out"]
    n_per = meta["n_per"]
    t_tiles = meta["t_tiles"]
    n_win = meta["n_win"]
    win_rows = meta["win_rows"]
    nchunk = meta["nchunk"]
    schedule = meta["schedule"]
    first_chunk = meta["first_chunk"]
    last_chunk = meta["last_chunk"]
    att = meta["att"]
    kc = f_in // P
    T = 3.0
    TT = t_tiles

    f32 = mybir.dt.float32
    bf16 = mybir.dt.bfloat16
    i16 = mybir.dt.int16
    AF = mybir.ActivationFunctionType
    OP = mybir.AluOpType

    nc = bacc.Bacc("TRN2", target_bir_lowering=False, debug=False,
                   num_devices=CORES, num_swdge_queues=NQ)

    xt_p = nc.declare_dram_parameter("xt", [f_in, n_per], bf16,
                                     isOutput=False)
    wcat_p = nc.declare_dram_parameter("wcat", [f_in, 3 * f_out], bf16,
                                       isOutput=False)
    avrep_p = nc.declare_dram_parameter("avrep", [P, 3 * f_out], bf16,
                                        isOutput=False)
    gidx_p = nc.declare_dram_parameter("gidx", [P, nchunk * 8], i16,
                                       isOutput=False)
    mask_p = nc.declare_dram_parameter("masks", [P, nchunk * P], bf16,
                                       isOutput=False)
    out_p = nc.declare_dram_parameter("out", [n_per, f_out], f32,
                                      isOutput=True)

    with tile.TileContext(nc) as tc:
        with tc.tile_pool(name="dram", bufs=1, space="DRAM") as dram_pool, \
             tc.tile_pool(name="static", bufs=1) as sp:
            hpart = [dram_pool.tile([n_per, f_out], bf16, name=f"hpart{b}")
                     for b in range(2)]
            hall = [dram_pool.tile([n, f_out], bf16, addr_space="Shared",
                                   name=f"hall{b}")
                    for b in range(2)]

            w_sb = sp.tile([P, kc, 3 * f_out], bf16, name="w_sb")
            av_sb = sp.tile([P, 3 * f_out], bf16, name="av_sb")
            gidx_sb = sp.tile([P, nchunk * 8], i16, name="gidx_sb")
            olall = sp.tile([P, TT * f_out], bf16, name="olall")
            ohall = sp.tile([P, TT * f_out], bf16, name="ohall")
            omlp = sp.tile([P, TT * f_out], bf16, name="omlp")

            nc.sync.dma_start(out=w_sb[:],
                              in_=wcat_p[:].rearrange("(k p) f -> p k f",
                                                      p=P))
            nc.sync.dma_start(out=av_sb[:], in_=avrep_p[:])
            nc.sync.dma_start(out=gidx_sb[:], in_=gidx_p[:])
            if n_per % P:
                nc.vector.memset(omlp[:], 0)

            # ---------------- Phase A: h = x @ W ----------------
            hpart_writes = []
            with tc.tile_pool(name="xtp", bufs=1) as xtp, \
                 tc.tile_pool(name="psA", bufs=2, space="PSUM") as psA, \
                 tc.tile_pool(name="hbp", bufs=3) as hbp:
                xt_sb = xtp.tile([P, kc, n_per], bf16, name="xt_sb")
                nc.sync.dma_start(
                    out=xt_sb[:],
                    in_=xt_p[:].rearrange("(k p) m -> p k m", p=P))
                for t in range(t_tiles):
                    m = min(P, n_per - t * P)
                    ph = psA.tile([P, 3 * f_out], f32, name="ph", tag="ph")
                    for k in range(kc):
                        nc.tensor.matmul(
                            out=ph[:m],
                            lhsT=xt_sb[:, k, t * P:t * P + m],
                            rhs=w_sb[:, k, :],
                            start=(k == 0), stop=(k == kc - 1))
                    hb = hbp.tile([P, 2 * f_out], bf16, name="hb", tag="hb")
                    nc.vector.tensor_copy(out=hb[:m], in_=ph[:m, 0:2 * f_out])
                    nc.scalar.activation(
                        out=omlp[:m, t * f_out:(t + 1) * f_out],
                        in_=ph[:m, 2 * f_out:3 * f_out], func=AF.Relu)
                    d0 = nc.sync.dma_start(out=hpart[0][t * P:t * P + m, :],
                                           in_=hb[:m, 0:f_out])
                    d1 = nc.sync.dma_start(out=hpart[1][t * P:t * P + m, :],
                                           in_=hb[:m, f_out:2 * f_out])
                    hpart_writes += [d0, d1]

            # ---------------- AllGather ----------------
            cc = []
            for b in range(2):
                cci = nc.gpsimd.collective_compute(
                    "AllGather", OP.bypass,
                    replica_groups=[list(range(CORES))],
                    ins=[hpart[b].opt()],
                    outs=[hall[b].opt()])
                for dw in hpart_writes:
                    add_dep_helper(cci.ins, dw.ins, True,
                                   reason="cc after hpart write")
                cc.append(cci)

            # -------- Phase C: gather + mask-matmul segment sum --------
            qn = 0
            with tc.tile_pool(name="gbp", bufs=5) as gbp, \
                 tc.tile_pool(name="mkp", bufs=5) as mkp, \
                 tc.tile_pool(name="psC", bufs=2, space="PSUM") as psC:
                for gi, grp in enumerate(schedule):
                    tiles = grp["tiles"]
                    tloc = len(tiles)
                    g0 = tiles[0]
                    fw = tloc * f_out
                    ps = [psC.tile([P, TG * f_out], f32, name=f"ps{b}",
                                   tag=f"ps{b}") for b in range(2)]
                    for seg in grp["segs"]:
                        b, w, off, S = seg["b"], seg["w"], seg["off"], seg["S"]
                        gb = gbp.tile([P, S * f_out], bf16, name="gb",
                                      tag="gb")
                        gat = nc.gpsimd.dma_gather(
                            out_ap=gb[:].rearrange("p (s f) -> p s f",
                                                   f=f_out),
                            in_ap=hall[b][w * win_rows:
                                          min((w + 1) * win_rows, n), :],
                            idxs_ap=gidx_sb[:, off * 8:(off + S) * 8],
                            num_idxs=S * P,
                            num_idxs_reg=S * P,
                            elem_size=f_out,
                            single_packet=False,
                            queue_num=qn % NQ)
                        qn += 1
                        add_dep_helper(gat.ins, cc[b].ins, True,
                                       reason="gather after allgather")
                        mk = mkp.tile([P, S * f_out], bf16, name="mk",
                                      tag="mk")
                        nc.sync.dma_start(
                            out=mk[:],
                            in_=mask_p[:, off * P:(off + S) * P])
                        for (ti, cid, t, k) in seg["items"]:
                            sl = cid - off
                            nc.tensor.matmul(
                                out=ps[b][:, ti * f_out:(ti + 1) * f_out],
                                lhsT=mk[:, sl * P:(sl + 1) * P],
                                rhs=gb[:, sl * f_out:(sl + 1) * f_out],
                                start=(cid == first_chunk[(gi, b)]),
                                stop=(cid == last_chunk[(gi, b)]),
                                skip_group_check=True)
                    nc.scalar.activation(
                        out=olall[:, g0 * f_out:g0 * f_out + fw],
                        in_=ps[0][:, :fw], func=AF.Relu)
                    nc.scalar.activation(
                        out=ohall[:, g0 * f_out:g0 * f_out + fw],
                        in_=ps[1][:, :fw], func=AF.Relu)

            # ---------------- attention epilogue (batched) ----------------
            with tc.tile_pool(name="epp", bufs=1) as epp:
                tmp = epp.tile([P, TT * f_out], bf16, name="tmp")
                lg = epp.tile([P, 3 * TT], f32, name="lg")
                srcs = [olall, ohall, omlp]
                for j in range(3):
                    a3d = av_sb[:, j * f_out:(j + 1) * f_out][:, None, :] \
                        .broadcast_to([P, TT, f_out])
                    nc.vector.tensor_tensor(
                        out=tmp[:].rearrange("p (t f) -> p t f", f=f_out),
                        in0=srcs[j][:].rearrange("p (t f) -> p t f", f=f_out),
                        in1=a3d, op=OP.mult)
                    nc.vector.tensor_reduce(
                        out=lg[:, j * TT:(j + 1) * TT],
                        in_=tmp[:].rearrange("p (t f) -> p t f", f=f_out),
                        axis=mybir.AxisListType.X, op=OP.add)
                sg = epp.tile([P, 3 * TT], f32, name="sg")
                nc.scalar.activation(out=sg[:], in_=lg[:], func=AF.Sigmoid)
                zt = epp.tile([P, 3 * TT], f32, name="zt")
                t2 = epp.tile([P, TT], f32, name="t2")
                for j in range(3):
                    zj = zt[:, j * TT:(j + 1) * TT]
                    nc.vector.tensor_scalar(
                        out=zj, in0=sg[:, 0:TT],
                        scalar1=float(att[0, j] / T), scalar2=None,
                        op0=OP.mult)
                    for k2 in (1, 2):
                        nc.vector.tensor_scalar(
                            out=t2[:], in0=sg[:, k2 * TT:(k2 + 1) * TT],
                            scalar1=float(att[k2, j] / T), scalar2=None,
                            op0=OP.mult)
                        nc.vector.tensor_tensor(out=zj, in0=zj, in1=t2[:],
                                                op=OP.add)
                et = epp.tile([P, 3 * TT], f32, name="et")
                nc.scalar.activation(out=et[:], in_=zt[:], func=AF.Exp)
                s3 = epp.tile([P, TT], f32, name="s3")
                nc.vector.tensor_tensor(out=s3[:], in0=et[:, 0:TT],
                                        in1=et[:, TT:2 * TT], op=OP.add)
                nc.vector.tensor_tensor(out=s3[:], in0=s3[:],
                                        in1=et[:, 2 * TT:3 * TT], op=OP.add)
                # rcp = 3/(sum e)  so that e*rcp = 3*att
                nc.vector.reciprocal(out=s3[:], in_=s3[:])
                nc.vector.tensor_scalar(out=s3[:], in0=s3[:], scalar1=3.0,
                                        scalar2=None, op0=OP.mult)
                at = epp.tile([P, 3 * TT], bf16, name="at")
                for j in range(3):
                    nc.vector.tensor_tensor(
                        out=at[:, j * TT:(j + 1) * TT],
                        in0=et[:, j * TT:(j + 1) * TT],
                        in1=s3[:], op=OP.mult)
                oo = epp.tile([P, TT * f_out], f32, name="oo")
                tmp2 = epp.tile([P, TT * f_out], f32, name="tmp2")
                for j in range(3):
                    dst = oo if j == 0 else tmp2
                    a3d = at[:, j * TT:(j + 1) * TT][:, :, None] \
                        .broadcast_to([P, TT, f_out])
                    nc.vector.tensor_tensor(
                        out=dst[:].rearrange("p (t f) -> p t f", f=f_out),
                        in0=srcs[j][:].rearrange("p (t f) -> p t f", f=f_out),
                        in1=a3d, op=OP.mult)
                    if j > 0:
                        nc.vector.tensor_tensor(out=oo[:], in0=oo[:],
                                                in1=tmp2[:], op=OP.add)
                # output: full tiles in one 3D DMA, ragged tail separately
                nfull = n_per // P
                if nfull:
                    nc.sync.dma_start(
                        out=out_p[0:nfull * P, :].rearrange(
                            "(t p) f -> p t f", p=P),
                        in_=oo[:, 0:nfull * f_out].rearrange(
                            "p (t f) -> p t f", f=f_out))
                if n_per % P:
                    m = n_per - nfull * P
                    nc.sync.dma_start(
                        out=out_p[nfull * P:, :],
                        in_=oo[:m, nfull * f_out:(nfull + 1) * f_out])
    nc.compile()
    return nc


# --------------------------------------------------------------------------
# Entry point
# --------------------------------------------------------------------------

def _solve(inputs, trace=False):
    from concourse.bass_utils import run_bass_kernel_spmd

    meta, in_maps = preprocess(**inputs)
    nc = build_graph(meta)
    res = run_bass_kernel_spmd(nc, in_maps, core_ids=list(range(CORES)),
                               trace=trace)
    out = np.concatenate([res.results[c]["out"] for c in range(CORES)],
                         axis=0)
    return out.astype(np.float32), res


def kernel(**inputs):
    out, _ = _solve(inputs, trace=False)
    return out
